# revision 2
# baseline (speedup 1.0000x reference)
"""Trainium2 Bass kernel for the kinematic bicycle-model rollout, v2.2.

Transposed layout: time on SBUF partitions (16 segs x 128), batch on the
free dim (1024/core). The x/y prefix sums over time become PE matmuls
with a triangular ones matrix (contraction over the partition dim),
eliminating the serial DVE scans of the v1 kernel.

Closed forms (host, fp64):
  speed_t = s0 + c_t,   c_t = sum_{i<t} DT*MAX_ACC*clip(a_i)
  yaw_t   = yaw0 + s0*A_t + Bv_t          (A, Bv cumsums of k_i)
  u       = yaw/2pi;  w = u - round(u) via fp32 magic-add on DVE
  x_t     = x0 + sum_{i<t} cdts0_i*cos(yaw_i),  cdts0 = DT*(c + s0)

Carry folding (kills the K=1 carry matmuls): the u/v pipeline runs on
host-shifted columns (A'[t] = A[t-1], ...) so the v supertiles hold
v_{t-1} at row p <-> t = 128s+p. Seg-start carries C_s = x_{128s}
(from per-group Msel selector chains) are DMA'd into row 0 of each
seg, and one INCLUSIVE triangular matmul per 512-col chunk then
yields x rows directly -- carry included. yaw rides the same shifted
tiles (output DMA shifted one row; row t=2047 patched with a [1, BL]
stt); speed uses unshifted c columns.

Engine split: DVE elementwise fp16 2x chain (u1/u/rnd/w/|w|/yaw/cdts0/
speed/vx/vy), ACT sin+cos (Sin LUT) + all PSUM evicts, PE Msel carry
chains + inclusive prefix matmuls, no GpSimd (its SBUF traffic starves
DVE: measured 12x slowdowns on overlapped ops).
"""

import math
import sys

sys.path.insert(0, "/opt/trn_rl_repo")

import numpy as np

import concourse.bacc as bacc
import concourse.mybir as mybir
import concourse.tile as tile
from concourse.bass_utils import run_bass_kernel_spmd

H = 2048
B = 8192
NCORES = 8
BL = B // NCORES          # batch per core (free dim)
P = 128                   # partitions = time rows per seg
NSEG = H // P             # 16 segs
NGRP = 4                  # carry groups
SPG = NSEG // NGRP        # segs per group = 4
DT = 0.05
WHEELBASE = 2.5
MAX_STEER = 0.5
MAX_ACC = 5000.0 / 1000.0

TWO_PI = 2.0 * math.pi
INV_2PI = 1.0 / TWO_PI
HALF_PI = 0.5 * math.pi
MAGIC = 12582912.0        # 1.5*2^23: fp32 x+MAGIC-MAGIC == round(x)

F32 = mybir.dt.float32
F16 = mybir.dt.float16
I16 = mybir.dt.int16
AFT = mybir.ActivationFunctionType
ALU = mybir.AluOpType

_CACHE = {}


def _build():
    nc = bacc.Bacc("TRN2", target_bir_lowering=False, debug=False)

    # colpack [128, 84] f32 (col s = values for t in seg s):
    #   0-15  A'/2pi (shifted)   16-31 Bv'/2pi (shifted)
    #   32-47 c (unshifted, speed)   48-63 c' (shifted, cdts0)
    #   col 64 row 0: A[2047] (yaw patch scalar)
    colpack = nc.declare_dram_parameter("colpack", [P, 84], F32, isOutput=False)
    u128 = nc.declare_dram_parameter("u128", [P, P], F16, isOutput=False)
    msel = nc.declare_dram_parameter("msel", [P, 114], F16, isOutput=False)
    # selrow [1, 10]: cols 0-4 ones (GC fold), cols 5-9 = [0,0,0,0,1]
    selrow = nc.declare_dram_parameter("selrow", [1, 16], F16, isOutput=False)
    s0row = nc.declare_dram_parameter("s0row", [1, BL], F16, isOutput=False)
    u0row2 = nc.declare_dram_parameter("u0row2", [1, 2 * BL], F16,
                                       isOutput=False)
    ybvrow = nc.declare_dram_parameter("ybvrow", [1, BL], F16, isOutput=False)
    x0row = nc.declare_dram_parameter("x0row", [1, BL], F16, isOutput=False)
    y0row = nc.declare_dram_parameter("y0row", [1, BL], F16, isOutput=False)
    ox = nc.declare_dram_parameter("ox", [H, BL], F16, isOutput=True)
    oy = nc.declare_dram_parameter("oy", [H, BL], F16, isOutput=True)
    oyaw = nc.declare_dram_parameter("oyaw", [H, BL], F16, isOutput=True)
    ospeed = nc.declare_dram_parameter("ospeed", [H, BL], F16, isOutput=True)

    W2 = 2 * BL            # pair free width
    HB = BL // 2           # psum bank width in f32

    with tile.TileContext(nc) as tc:
        with (
            tc.tile_pool(name="const", bufs=1) as constp,
            tc.tile_pool(name="gw", bufs=2) as gw,
            tc.tile_pool(name="stage", bufs=3) as stg,
            tc.tile_pool(name="carry", bufs=2) as cp,
            tc.tile_pool(name="psC", bufs=1, space="PSUM") as psC,
            tc.tile_pool(name="psXY", bufs=2, space="PSUM") as psXY,
        ):
            # order matters: the first DVE op (u1) needs s0_bc + colp only
            s0_bc = constp.tile([P, BL], F16)
            nc.sync.dma_start(out=s0_bc[:],
                              in_=s0row[0, None, :].to_broadcast((P, BL)))
            colp = constp.tile([P, 84], F32)
            nc.sync.dma_start(out=colp[:], in_=colpack[:])
            u0_bc2 = constp.tile([P, W2], F16)
            nc.sync.dma_start(out=u0_bc2[:],
                              in_=u0row2[0, None, :].to_broadcast((P, W2)))
            u128sb = constp.tile([P, P], F16)
            nc.sync.dma_start(out=u128sb[:], in_=u128[:])
            mselsb = constp.tile([P, 114], F16)
            nc.sync.dma_start(out=mselsb[:], in_=msel[:])
            selsb = constp.tile([1, 16], F16)
            nc.sync.dma_start(out=selsb[:], in_=selrow[:])
            s0r_sb = constp.tile([1, BL], F16)
            nc.sync.dma_start(out=s0r_sb[:], in_=s0row[:])
            ybv_sb = constp.tile([1, BL], F16)
            nc.sync.dma_start(out=ybv_sb[:], in_=ybvrow[:])
            halfpi_col = constp.tile([P, 1], F32)
            nc.vector.memset(halfpi_col[:], HALF_PI)
            mask_col = constp.tile([P, 1], I16)
            nc.vector.memset(mask_col[:], 0x7FFF)

            # group-carry rows: gc[g] [1, BL] fp16 (x at group-g start)
            gcx = [cp.tile([1, BL], F16, tag=f"gcx{g}", name=f"gcx{g}", bufs=1)
                   for g in range(3)]
            gcy = [cp.tile([1, BL], F16, tag=f"gcy{g}", name=f"gcy{g}", bufs=1)
                   for g in range(3)]
            nc.sync.dma_start(out=gcx[0][:], in_=x0row[:])
            nc.sync.dma_start(out=gcy[0][:], in_=y0row[:])

            vx_super = constp.tile([P, NSEG * BL], F16)
            vy_super = constp.tile([P, NSEG * BL], F16)

            def phase1_pair(p):
                """Elementwise chain for segs (2p, 2p+1), shifted-t.

                Critical path first: u1 -> u -> rnd -> w -> wa feed the ACT
                sin/cos; cdts0/speed/yaw fill DVE while ACT runs the LUTs.
                """
                gsl = slice(2 * p * BL, (2 * p + 2) * BL)
                u1 = gw.tile([P, W2], F16, tag="u1", bufs=1)
                for half in range(2):
                    s = 2 * p + half
                    hsl = slice(half * BL, (half + 1) * BL)
                    nc.vector.tensor_scalar(
                        out=u1[:, hsl], in0=s0_bc[:],
                        scalar1=colp[:, s : s + 1],
                        scalar2=colp[:, 16 + s : 17 + s],
                        op0=ALU.mult, op1=ALU.add,
                    )
                u = gw.tile([P, W2], F16, tag="u")
                nc.vector.tensor_tensor(out=u[:], in0=u1[:], in1=u0_bc2[:],
                                        op=ALU.add)
                rnd = gw.tile([P, W2], F16, tag="rnd", bufs=1)
                nc.vector.tensor_scalar(out=rnd[:], in0=u[:],
                                        scalar1=float(MAGIC),
                                        scalar2=float(MAGIC),
                                        op0=ALU.add, op1=ALU.subtract)
                w = gw.tile([P, W2], F16, tag="w")
                nc.vector.tensor_tensor(out=w[:], in0=u[:], in1=rnd[:],
                                        op=ALU.subtract)
                wa = gw.tile([P, W2], F16, tag="wa", bufs=1)
                nc.vector.tensor_scalar(out=wa[:].bitcast(I16),
                                        in0=w[:].bitcast(I16),
                                        scalar1=mask_col[:], scalar2=None,
                                        op0=ALU.bitwise_and)
                sint = gw.tile([P, W2], F16, tag="sin")
                nc.scalar.activation(out=sint[:], in_=w[:], func=AFT.Sin,
                                     scale=TWO_PI)
                cost = gw.tile([P, W2], F16, tag="cos")
                nc.scalar.activation(out=cost[:], in_=wa[:], func=AFT.Sin,
                                     scale=-TWO_PI, bias=halfpi_col[:])

                # filler work while ACT runs the LUTs
                cdts0 = gw.tile([P, W2], F16, tag="cdts0")
                for half in range(2):
                    s = 2 * p + half
                    hsl = slice(half * BL, (half + 1) * BL)
                    nc.vector.tensor_scalar(
                        out=cdts0[:, hsl], in0=s0_bc[:],
                        scalar1=colp[:, 48 + s : 49 + s], scalar2=float(DT),
                        op0=ALU.add, op1=ALU.mult,
                    )
                yawt = gw.tile([P, W2], F16, tag="yaw")
                nc.vector.tensor_scalar(out=yawt[:], in0=u[:],
                                        scalar1=float(TWO_PI), scalar2=None,
                                        op0=ALU.mult)
                if p == 0:
                    nc.gpsimd.dma_start(out=oyaw[0:127, :],
                                        in_=yawt[1:P, 0:BL])
                    nc.gpsimd.dma_start(out=oyaw[127 : 2 * P - 1, :],
                                        in_=yawt[:, BL:W2])
                else:
                    nc.gpsimd.dma_start(
                        out=oyaw[2 * p * P - 1 : (2 * p + 2) * P - 1,
                                 :].rearrange("(k q) b -> q k b", k=2),
                        in_=yawt[:, :])
                if p < 6:
                    emit_speed(p)

                nc.vector.tensor_tensor(out=vx_super[:, gsl], in0=cdts0[:],
                                        in1=cost[:], op=ALU.mult)
                nc.vector.tensor_tensor(out=vy_super[:, gsl], in0=cdts0[:],
                                        in1=sint[:], op=ALU.mult)

            def emit_speed(p):
                spd = gw.tile([P, W2], F16, tag="spd")
                for half in range(2):
                    s = 2 * p + half
                    hsl = slice(half * BL, (half + 1) * BL)
                    nc.vector.tensor_scalar(
                        out=spd[:, hsl], in0=s0_bc[:],
                        scalar1=colp[:, 32 + s : 33 + s], scalar2=None,
                        op0=ALU.add,
                    )
                nc.gpsimd.dma_start(
                    out=ospeed[2 * p * P : (2 * p + 2) * P, :].rearrange(
                        "(k q) b -> q k b", k=2),
                    in_=spd[:])

            SPGS = [6, 6, 3]       # group 3 = seg 15, chainless
            MOFF = [0, 49, 98]     # msel col-block offsets per group
            GS0 = [0, 6, 12, 15]   # group start segs

            def msel_mm(g, j, cg_ps, vsuper, start):
                s = GS0[g] + j
                ng = SPGS[g]
                mj = ng if (j == 0 and g >= 1) else j   # boundary variant
                off = MOFF[g] + mj * (ng + 1)
                for ch in range(2):
                    csl = slice(ch * HB, (ch + 1) * HB)
                    vsl = slice(s * BL + ch * HB, s * BL + (ch + 1) * HB)
                    nc.tensor.matmul(cg_ps[0 : ng + 1, csl],
                                     mselsb[:, off : off + ng + 1],
                                     vsuper[:, vsl], start=start, stop=False)

            def close_group(g, cg_ps, gc):
                ng = SPGS[g]
                for ch in range(2):
                    csl = slice(ch * HB, (ch + 1) * HB)
                    nc.tensor.matmul(cg_ps[0 : ng + 1, csl],
                                     selsb[:, 0 : ng + 1],
                                     gc[:, csl], start=False, stop=True)

            def phase2_pair(s0_, vsuper, out_dram):
                """Prefix + evict for segs (s0_, s0_+1) of one output."""
                osb = stg.tile([P, W2], F16, tag="osb")
                for k in range(2):
                    s = s0_ + k
                    ps = psXY.tile([P, BL], F32, tag="xyps")
                    for ch in range(2):
                        csl = slice(ch * HB, (ch + 1) * HB)
                        vsl = slice(s * BL + ch * HB, s * BL + (ch + 1) * HB)
                        nc.tensor.matmul(ps[:, csl], u128sb[:],
                                         vsuper[:, vsl], start=True,
                                         stop=True)
                    nc.scalar.activation(out=osb[:, k * BL : (k + 1) * BL],
                                         in_=ps[:], func=AFT.Copy)
                nc.sync.dma_start(
                    out=out_dram[s0_ * P : (s0_ + 2) * P, :].rearrange(
                        "(k p) b -> p k b", k=2),
                    in_=osb[:])

            def phase2_seg(s, vsuper, out_dram, evict_dve):
                """Single-seg prefix for the tail; evict on ACT or DVE."""
                ps = psXY.tile([P, BL], F32, tag="xyps")
                for ch in range(2):
                    csl = slice(ch * HB, (ch + 1) * HB)
                    vsl = slice(s * BL + ch * HB, s * BL + (ch + 1) * HB)
                    nc.tensor.matmul(ps[:, csl], u128sb[:], vsuper[:, vsl],
                                     start=True, stop=True)
                osb = stg.tile([P, BL], F16, tag="osbs")
                if evict_dve:
                    nc.vector.tensor_copy(out=osb[:], in_=ps[:])
                else:
                    nc.scalar.activation(out=osb[:], in_=ps[:], func=AFT.Copy)
                nc.sync.dma_start(out=out_dram[s * P : (s + 1) * P, :],
                                  in_=osb[:])

            def finish_group(g):
                """Close chains, evict C, seed row-0 carries, run phase 2."""
                ng = SPGS[g]
                cgx_ps, cgy_ps = cg_live[g]
                close_group(g, cgx_ps, gcx[g])
                close_group(g, cgy_ps, gcy[g])
                cgx_sb = cp.tile([7, BL], F16, tag="cgxsb")
                nc.scalar.activation(out=cgx_sb[0 : ng + 1, :],
                                     in_=cgx_ps[0 : ng + 1, :], func=AFT.Copy)
                cgy_sb = cp.tile([7, BL], F16, tag="cgysb")
                nc.scalar.activation(out=cgy_sb[0 : ng + 1, :],
                                     in_=cgy_ps[0 : ng + 1, :], func=AFT.Copy)
                if g + 1 < len(gcx):
                    nc.sync.dma_start(out=gcx[g + 1][:],
                                      in_=cgx_sb[ng : ng + 1, :])
                    nc.sync.dma_start(out=gcy[g + 1][:],
                                      in_=cgy_sb[ng : ng + 1, :])
                gsl0 = slice(GS0[g] * BL, (GS0[g] + ng) * BL)
                nc.sync.dma_start(out=vx_super[0:1, gsl0],
                                  in_=cgx_sb[0:ng, :])
                nc.sync.dma_start(out=vy_super[0:1, gsl0],
                                  in_=cgy_sb[0:ng, :])
                return cgx_sb, cgy_sb

            def phase2_group(g):
                ng = SPGS[g]
                for j in range(0, ng - 1, 2):
                    s = GS0[g] + j
                    phase2_pair(s, vx_super, ox)
                    phase2_pair(s, vy_super, oy)
                if ng % 2:
                    s = GS0[g] + ng - 1
                    phase2_seg(s, vx_super, ox, evict_dve=False)
                    phase2_seg(s, vy_super, oy, evict_dve=True)

            # ---- main software-pipelined loop ----
            # pairs 0-2 -> group 0 (segs 0-5); pairs 3-5 -> group 1;
            # pair 6 + seg14 of pair 7 -> group 2; seg 15 chainless.
            cg_live = {}

            def new_chain(g):
                ng = SPGS[g]
                cgx_ps = psC.tile([7, BL], F32, tag="cgx", name=f"cgx_ps{g}")
                cgy_ps = psC.tile([7, BL], F32, tag="cgy", name=f"cgy_ps{g}")
                cg_live[g] = (cgx_ps, cgy_ps)

            for p in range(8):
                phase1_pair(p)
                segs = (2 * p, 2 * p + 1)
                for s in segs:
                    g = 0 if s < 6 else (1 if s < 12 else (2 if s < 15 else 3))
                    if g == 3:
                        continue
                    j = s - GS0[g]
                    if j == 0:
                        new_chain(g)
                    cgx_ps, cgy_ps = cg_live[g]
                    msel_mm(g, j, cgx_ps, vx_super, j == 0)
                    msel_mm(g, j, cgy_ps, vy_super, j == 0)
                # stagger: finish group g-1 while group g's pairs stream
                if p == 3:
                    finish_group(0)
                    phase2_group(0)
                elif p == 6:
                    finish_group(1)
                    phase2_group(1)
            emit_speed(6)
            emit_speed(7)
            cgx_sb3, cgy_sb3 = finish_group(2)
            # seg 15 carry: C_15 = GC_3 + v_1919 (tile-15 row 0, not yet
            # covered by any chain). GC row sits at partition 3 -- bounce
            # it to partition 0 by DMA (engines need quadrant-aligned base)
            gc3x = cp.tile([1, BL], F16, tag="gc3x", bufs=1)
            nc.sync.dma_start(out=gc3x[:], in_=cgx_sb3[3:4, :])
            gc3y = cp.tile([1, BL], F16, tag="gc3y", bufs=1)
            nc.sync.dma_start(out=gc3y[:], in_=cgy_sb3[3:4, :])
            c15x = cp.tile([1, BL], F16, tag="c15x", bufs=1)
            nc.vector.tensor_tensor(out=c15x[:], in0=gc3x[:],
                                    in1=vx_super[0:1, 15 * BL : 16 * BL],
                                    op=ALU.add)
            c15y = cp.tile([1, BL], F16, tag="c15y", bufs=1)
            nc.vector.tensor_tensor(out=c15y[:], in0=gc3y[:],
                                    in1=vy_super[0:1, 15 * BL : 16 * BL],
                                    op=ALU.add)
            nc.sync.dma_start(out=vx_super[0:1, 15 * BL : 16 * BL],
                              in_=c15x[:])
            nc.sync.dma_start(out=vy_super[0:1, 15 * BL : 16 * BL],
                              in_=c15y[:])
            phase2_group(2)
            phase2_seg(15, vx_super, ox, evict_dve=False)
            phase2_seg(15, vy_super, oy, evict_dve=True)

            # yaw row t=2047 patch: yaw = A[2047]*s0 + (yaw0 + Bv[2047])
            ypr = stg.tile([1, BL], F16, tag="ypr", bufs=1)
            nc.vector.scalar_tensor_tensor(
                out=ypr[:], in0=s0r_sb[:], scalar=colp[0:1, 64:65],
                in1=ybv_sb[:], op0=ALU.mult, op1=ALU.add,
            )
            nc.sync.dma_start(out=oyaw[H - 1 : H, :], in_=ypr[:])

    nc.finalize()
    return nc


def _host_precompute(accel, steering):
    a = np.clip(accel.astype(np.float64), -1.0, 1.0)
    dv = DT * MAX_ACC * a
    c = np.concatenate([[0.0], np.cumsum(dv)[: H - 1]])
    st = np.clip(steering.astype(np.float64), -MAX_STEER, MAX_STEER)
    k = np.tan(st) / WHEELBASE * DT
    A = np.concatenate([[0.0], np.cumsum(k)[: H - 1]])
    Bv = np.concatenate([[0.0], np.cumsum(c * k)[: H - 1]])

    def shift(v):   # v'[t] = v[t-1]; v'[0] padded (row 0 overwritten/skipped)
        return np.concatenate([[v[0]], v[:-1]])

    def cols(v):    # [H] -> [128, 16] (col s = seg s)
        return np.ascontiguousarray(v.reshape(NSEG, P).T)

    colpack = np.zeros((P, 84), np.float64)
    colpack[:, 0:16] = cols(shift(A) * INV_2PI)
    colpack[:, 16:32] = cols(shift(Bv) * INV_2PI)
    colpack[:, 32:48] = cols(c)
    colpack[:, 48:64] = cols(shift(c))
    colpack[0, 64] = A[H - 1]
    colpack = colpack.astype(np.float32)

    u128 = np.triu(np.ones((P, P), np.float16), k=0)  # inclusive: 1 if i<=j
    SPGS = [6, 6, 3]
    MOFF = [0, 49, 98]
    msel = np.zeros((P, 114), np.float16)
    for g, ng in enumerate(SPGS):
        for j in range(ng):
            off = MOFF[g] + j * (ng + 1)
            for j2 in range(ng):
                msel[0, off + j2] = 1.0 if (1 <= j <= j2) else 0.0
                msel[1:, off + j2] = 1.0 if j < j2 else 0.0
            msel[0, off + ng] = 1.0 if j >= 1 else 0.0
            msel[1:, off + ng] = 1.0
        # boundary variant (block index ng): j=0 with row 0 all-ones
        off = MOFF[g] + ng * (ng + 1)
        msel[:, off : off + ng + 1] = msel[:, MOFF[g] : MOFF[g] + ng + 1]
        msel[0, off : off + ng + 1] = 1.0
    selrow = np.zeros((1, 16), np.float16)
    selrow[0, 0:7] = 1.0
    return colpack, u128, msel, selrow, A, Bv


def _install_ntff_shim():
    import types

    import antenv

    if hasattr(antenv, "axon_hooks"):
        return
    mod = types.ModuleType("antenv.axon_hooks")
    holder = [None]
    mod.set_axon_ntff_profile_hook = lambda h: holder.__setitem__(0, h)
    mod.get_axon_ntff_profile_hook = lambda: holder[0]
    sys.modules["antenv.axon_hooks"] = mod
    antenv.axon_hooks = mod
    from trn_agent_boot.trn_boot import _ntff_profile_via_ctypes

    mod.set_axon_ntff_profile_hook(
        _ntff_profile_via_ctypes("/opt/axon/libaxon_pjrt.so")
    )


def run(start_x, start_y, start_yaw, start_speed, accel, steering, trace=False,
        tmpdir=None):
    if "nc" not in _CACHE:
        _CACHE["nc"] = _build()
    nc = _CACHE["nc"]
    if trace:
        _install_ntff_shim()

    start_x = np.asarray(start_x, dtype=np.float32)
    start_y = np.asarray(start_y, dtype=np.float32)
    start_yaw = np.asarray(start_yaw, dtype=np.float32)
    start_speed = np.asarray(start_speed, dtype=np.float32)
    colpack, u128, msel, selrow, A, Bv = _host_precompute(
        np.asarray(accel), np.asarray(steering))

    in_maps = []
    for i in range(NCORES):
        sl = slice(i * BL, (i + 1) * BL)
        s0 = start_speed[sl].astype(np.float64)
        yaw0 = start_yaw[sl].astype(np.float64)
        u0 = (yaw0 * INV_2PI).astype(np.float16)
        in_maps.append({
            "colpack": colpack, "u128": u128, "msel": msel, "selrow": selrow,
            "s0row": s0.astype(np.float16)[None, :],
            "u0row2": np.tile(u0, 2)[None, :],
            "ybvrow": (yaw0 + Bv[H - 1]).astype(np.float16)[None, :],
            "x0row": start_x[sl].astype(np.float16)[None, :],
            "y0row": start_y[sl].astype(np.float16)[None, :],
        })

    res = run_bass_kernel_spmd(nc, in_maps, core_ids=list(range(NCORES)),
                               trace=trace, tmpdir=tmpdir)

    outs = []
    for key in ("ox", "oy", "oyaw", "ospeed"):
        full = np.concatenate([res.results[i][key] for i in range(NCORES)],
                              axis=1)
        outs.append(np.ascontiguousarray(full.astype(np.float32)))
    return tuple(outs), res


def kernel(start_x, start_y, start_yaw, start_speed, accel, steering):
    outs, _ = run(start_x, start_y, start_yaw, start_speed, accel, steering)
    return outs
